# revision 7
# baseline (speedup 1.0000x reference)
"""Trainium2 Bass kernel for nn_DelayExpansionLayer (histogram_binning).

Computation: per-channel mean of layer_output [64,256,56,56] over (B,H,W),
round to 1e-6, nearest-key lookup in a sorted 1024-entry table, max over
channels, scale by (in_ch*out_ch)/512, broadcast to (56,56).

Strategy (data-parallel over batch, 8 NeuronCores):
  - Each core gets 8 batches = [8,256,56,56] (25.7 MB) and computes
    per-channel partial sums [256] on-device (DMA-bound reduction).
  - Host combines the 8 partial-sum vectors (the tiny [C] all-reduce),
    then does the O(C+K) lookup/max/broadcast epilogue.

Per-core device kernel (raw bass, manual semaphores):
  input  x [8, 128, 2, 3136] f32  (batch, partition, chan-pair, spatial).
  HW DGE sprays a DMA's outermost (partition) dim round-robin over the 16
  SDMA engines; engine 15 (E79) also runs the dynamic queues' bookkeeping
  and moves bytes ~20% slower, so it paces any 128-wide stream. Batches
  0-5 load as full 3.2MB 128-wide DMAs (all 16 engines); batches 6-7 are
  issued as 15-partition-wide DMAs (engines 0-14 only) with the 8-wide
  partition remainder issued from the scalar engine's separate HW queue
  (qScalarDynamicHW), so the late stream avoids E79 entirely and the
  byte-load is balanced (~1.66MB on engines 0-7 vs 1.2MB on E79).
  Reduction is split across DVE (tensor_reduce) and ACT (activation-Copy
  accum); late batches are j-split so the last reduces are short. The
  [128,2,10] partial sums leave via one DMA on the scalar queue (empty
  FIFO, bypasses the input stream). Channel c = 2*p + j.
"""

import sys
import types

import numpy as np

N_CORES = 8
B_FULL, C, H, W = 64, 256, 56, 56
HW = H * W
B_LOCAL = B_FULL // N_CORES
SCALE_DENOM = 32 * 16

# Set by a test harness to enable NTFF tracing of the SPMD run.
TRACE = False
TRACE_TMPDIR = None
LAST_RESULTS = None

_CACHE = {}

# 15-wide partition ranges (engines 0-14) + the 8-wide remainder (engines
# 0-7, issued from the scalar queue).
P15 = [(p, p + 15) for p in range(0, 120, 15)]
PREM = (120, 128)


def _ensure_axon_hooks_shim():
    """bass_utils' axon trace path imports antenv.axon_hooks; provide a
    no-op shim when the environment's antenv package lacks it."""
    try:
        import antenv.axon_hooks  # noqa: F401
        return
    except ImportError:
        pass

    mod = types.ModuleType("antenv.axon_hooks")
    _hook = [None]
    mod.set_axon_ntff_profile_hook = lambda h: _hook.__setitem__(0, h)
    mod.get_axon_ntff_profile_hook = lambda: _hook[0]
    sys.modules["antenv.axon_hooks"] = mod
    try:
        import antenv

        antenv.axon_hooks = mod
    except ImportError:
        pass


def _build():
    if "nc" in _CACHE:
        return _CACHE["nc"]
    import concourse.bass as bass
    from concourse import mybir

    nc = bass.Bass(
        "TRN2",
        target_bir_lowering=False,
        debug=False,
        enable_asserts=False,
        num_devices=N_CORES,
    )
    f32 = mybir.dt.float32
    x = nc.dram_tensor("x", [B_LOCAL, 128, 2, HW], f32, kind="ExternalInput").ap()
    out = nc.dram_tensor("out", [128, 2, 10], f32, kind="ExternalOutput").ap()

    # 6 batch slots (b0-b3 -> s0-s3, b4 -> s5 fresh, b5 -> s0 reused,
    # b6 -> s4 fresh) + exact-size tail buffers for b7's tapered chunks.
    slots = [
        nc.alloc_sbuf_tensor(f"slot{i}", [128, 2, HW], f32).ap() for i in range(6)
    ]
    t0 = nc.alloc_sbuf_tensor("t0", [128, HW], f32).ap()
    t1 = nc.alloc_sbuf_tensor("t1", [128, 1568], f32).ap()
    t2 = nc.alloc_sbuf_tensor("t2", [128, 784], f32).ap()
    t3 = nc.alloc_sbuf_tensor("t3", [128, 784], f32).ap()
    stats = nc.alloc_sbuf_tensor("stats", [128, 2, 10], f32).ap()

    # b7 taper chunks: (j, s0, s1, dest tile, stats col)
    TAIL = (
        (0, 0, HW, t0, 7),
        (1, 0, 1568, t1, 7),
        (1, 1568, 2352, t2, 8),
        (1, 2352, HW, t3, 9),
    )

    with (
        nc.Block(no_gpsimd_drain=True) as block,
        nc.semaphore("ds0") as ds0,
        nc.semaphore("ds1") as ds1,
        nc.semaphore("ds2") as ds2,
        nc.semaphore("ds3") as ds3,
        nc.semaphore("ds4") as ds4,
        nc.semaphore("ds5") as ds5,
        nc.semaphore("ds6") as ds6,
        nc.semaphore("dt0") as dt0,
        nc.semaphore("dt1") as dt1,
        nc.semaphore("dt2") as dt2,
        nc.semaphore("dt3") as dt3,
        nc.semaphore("vd") as vd,
        nc.semaphore("ad") as ad,
        nc.semaphore("od") as od,
    ):
        dt = [dt0, dt1, dt2, dt3]

        @block.sync
        def _(sync: bass.BassEngine):
            # b0-b3 full-width into s0-s3, no deps
            for b, sem in ((0, ds0), (1, ds1), (2, ds2), (3, ds3)):
                sync.dma_start(out=slots[b][:], in_=x[b]).then_inc(sem, 16)
            # b4 full-width into fresh s5
            sync.dma_start(out=slots[5][:], in_=x[4]).then_inc(ds5, 16)
            # b5 j-split into s0 (needs b0's DVE reduce)
            sync.wait_ge(vd, 1)
            for j in range(2):
                sync.dma_start(
                    out=slots[0][:, j, :], in_=x[5, :, j, :]
                ).then_inc(ds0, 16)
            # b6 15-wide into fresh s4 (remainder comes from scalar queue).
            # j0 and j1 count on separate sems: the scalar-queue remainders
            # complete early, so a shared count could fire before the last
            # 15-wide DMA lands.
            for j, sem in ((0, ds4), (1, ds6)):
                for p0, p1 in P15:
                    sync.dma_start(
                        out=slots[4][p0:p1, j, :], in_=x[6, p0:p1, j, :]
                    ).then_inc(sem, 16)
            # b7 taper chunks, 15-wide, fresh tiles
            for i, (j, s0, s1, tile, _k) in enumerate(TAIL):
                w = s1 - s0
                for p0, p1 in P15:
                    sync.dma_start(
                        out=tile[p0:p1, 0:w], in_=x[7, p0:p1, j, s0:s1]
                    ).then_inc(dt[i], 16)
            sync.wait_ge(od, 16)

        @block.vector
        def _(vector: bass.BassEngine):
            # b0, b2 whole-slot reduces
            for b, slot, sem, thr in ((0, slots[0], ds0, 16), (2, slots[2], ds2, 16)):
                vector.wait_ge(sem, thr)
                vector.reduce_sum(
                    stats[:, :, b : b + 1], slot[:], axis=mybir.AxisListType.X
                ).then_inc(vd, 1)
            # b6 per-j reduces (each sem counts exactly its 9 DMAs)
            for j, sem in ((0, ds4), (1, ds6)):
                vector.wait_ge(sem, 144)
                vector.reduce_sum(
                    stats[:, j, 6:7],
                    slots[4][:, j, :],
                    axis=mybir.AxisListType.X,
                ).then_inc(vd, 1)
            # b7 chunks c0 (j0 full) and c1 (j1 0:1568)
            for i in (0, 1):
                j, s0, s1, tile, k = TAIL[i]
                vector.wait_ge(dt[i], 144)
                vector.reduce_sum(
                    stats[:, j, k : k + 1],
                    tile[:, 0 : s1 - s0],
                    axis=mybir.AxisListType.X,
                ).then_inc(vd, 1)

        @block.scalar
        def _(scalar: bass.BassEngine):
            # 8-wide partition remainders for b6 + b7 chunks, enqueued up
            # front on the (empty) scalar HW queue -- fresh tiles, no deps.
            p0, p1 = PREM
            for j, sem in ((0, ds4), (1, ds6)):
                scalar.dma_start(
                    out=slots[4][p0:p1, j, :], in_=x[6, p0:p1, j, :]
                ).then_inc(sem, 16)
            for i, (j, s0, s1, tile, _k) in enumerate(TAIL):
                w = s1 - s0
                scalar.dma_start(
                    out=tile[p0:p1, 0:w], in_=x[7, p0:p1, j, s0:s1]
                ).then_inc(dt[i], 16)

            # ACT accum reduces: b1, b3, b4 pairs; b5 per-j; c2, c3
            for b, slot, sem, thr in (
                (1, slots[1], ds1, 16),
                (3, slots[3], ds3, 16),
                (4, slots[5], ds5, 16),
            ):
                scalar.wait_ge(sem, thr)
                for j in range(2):
                    ins = scalar.activation(
                        slot[:, j, :],
                        slot[:, j, :],
                        mybir.ActivationFunctionType.Copy,
                        accum_out=stats[:, j, b : b + 1],
                    )
                    if j == 1:
                        ins.then_inc(ad, 1)
            for j, thr in ((0, 32), (1, 48)):
                scalar.wait_ge(ds0, thr)
                scalar.activation(
                    slots[0][:, j, :],
                    slots[0][:, j, :],
                    mybir.ActivationFunctionType.Copy,
                    accum_out=stats[:, j, 5:6],
                ).then_inc(ad, 1)
            for i in (2, 3):
                j, s0, s1, tile, k = TAIL[i]
                scalar.wait_ge(dt[i], 144)
                scalar.activation(
                    tile[:, 0 : s1 - s0],
                    tile[:, 0 : s1 - s0],
                    mybir.ActivationFunctionType.Copy,
                    accum_out=stats[:, j, k : k + 1],
                ).then_inc(ad, 1)
            # single out-DMA on the scalar queue (bypasses input FIFO).
            # ad>=7 orders it after ACT's last accumulator writeback (the
            # inc fires post-writeback); vd>=6 after DVE's last reduce.
            scalar.wait_ge(ad, 7)
            scalar.wait_ge(vd, 6)
            scalar.dma_start(out=out[:], in_=stats[:]).then_inc(od, 16)

    _CACHE["nc"] = nc
    return nc


def kernel(layer_output, delay_keys, delay_values, in_channels, out_channels):
    global LAST_RESULTS
    _ensure_axon_hooks_shim()
    from concourse.bass_utils import run_bass_kernel_spmd

    x = np.ascontiguousarray(np.asarray(layer_output, dtype=np.float32))
    assert x.shape == (B_FULL, C, H, W), x.shape
    # shard over batch; view channels as (partition, pair): c = 2*p + j
    xr = x.reshape(N_CORES, B_LOCAL, 128, 2, HW)
    in_maps = [{"x": xr[k]} for k in range(N_CORES)]

    nc = _build()
    kwargs = {}
    if TRACE:
        kwargs.update(trace=True, tmpdir=TRACE_TMPDIR)
    res = run_bass_kernel_spmd(nc, in_maps, core_ids=list(range(N_CORES)), **kwargs)
    LAST_RESULTS = res

    # tiny [C] all-reduce of the per-core partial sums
    parts = np.stack(
        [res.results[k]["out"] for k in range(N_CORES)]
    )  # [8, 128, 2, 10]; j=0 valid cols 0..7, j=1 valid cols 0..9
    s0 = parts[:, :, 0, 0:8].sum(axis=(0, 2), dtype=np.float32)
    s1 = parts[:, :, 1, 0:10].sum(axis=(0, 2), dtype=np.float32)
    sums = np.stack([s0, s1], axis=1).reshape(C)  # c = 2p+j
    means = sums / np.float32(B_FULL * HW)
    means = np.round(means * np.float32(1e6)) / np.float32(1e6)

    keys = np.asarray(delay_keys, dtype=np.float32)
    values = np.asarray(delay_values, dtype=np.float32)
    K = keys.shape[0]
    idx = np.searchsorted(keys, means)
    lo = np.clip(idx - 1, 0, K - 1)
    hi = np.clip(idx, 0, K - 1)
    pick_hi = np.abs(keys[hi] - means) < np.abs(keys[lo] - means)
    nearest = np.where(pick_hi, hi, lo)
    merged = np.float32(values[nearest].max())

    scale = np.float32(
        (int(np.asarray(in_channels)) * int(np.asarray(out_channels))) / SCALE_DENOM
    )
    return np.full((H, W), merged, dtype=np.float32) * scale
